# revision 5
# baseline (speedup 1.0000x reference)
"""Trainium2 Bass kernel for CLSProcess: diagonal linear recurrence
state_t = y_t * state_{t-1} + x_t * z_t over [B=8, T=4096, units=1024].

Sharding: batch across the 8 cores (one batch element per core); the
recurrence is handled per-core with a chunked scan:
  - time is cut into 32 blocks of L=128 steps (partition dim = time)
  - per block, the decay matrix M[t,s] = prod_{r=s+1..t} y_r (0 for s>t)
    is built EXACTLY (no log/exp) with one DVE tensor_tensor_scan over the
    identity matrix: state_s(t) = y_t*state + I[s==t]  =>  out[s,t] = M[t,s],
    which is already the lhsT layout the PE matmul wants
  - block output = M @ (x*z)  (PE matmul, fp32)  +  carry term
  - carry term: engines can only address partition bases {0,32,64,96}, so
    instead of extracting row 127 of the previous block we build
    sel[s,t] = I[s==127] * p_t  (rank-1 PE matmul e127 ⊗ p_row, where
    p_t = prod_{r=block_start..t} y_r = y_0 * M[t,0]) and accumulate
    sel^T @ prev_out into the same PSUM, which equals p_t * prev_state.
"""

import numpy as np

import concourse.bacc as bacc
import concourse.bass as bass
import concourse.mybir as mybir
import concourse.tile as tile
from concourse.bass_utils import run_bass_kernel_spmd

B = 8
T = 4096
F = 1026
U = 1024
L = 128
f32 = mybir.dt.float32


def build_nc(t_total: int = T) -> bass.Bass:
    nb = t_total // L
    nc = bacc.Bacc()
    inp = nc.dram_tensor("inp", [t_total, F], f32, kind="ExternalInput")
    out = nc.dram_tensor("out", [t_total, U], f32, kind="ExternalOutput")
    ident_d = nc.inline_tensor(np.eye(L, dtype=np.float32), name="ident")
    ones_d = nc.inline_tensor(np.ones((1, L), dtype=np.float32), name="ones1")
    e127_np = np.zeros((1, L), dtype=np.float32)
    e127_np[0, L - 1] = 1.0
    e127_d = nc.inline_tensor(e127_np, name="e127")

    with tile.TileContext(nc) as tc:
        with (
            tc.tile_pool(name="const", bufs=1) as constp,
            tc.tile_pool(name="inpool", bufs=4) as inpool,
            tc.tile_pool(name="upool", bufs=2) as upool,
            tc.tile_pool(name="mpool", bufs=2) as mpool,
            tc.tile_pool(name="rowpool", bufs=2) as rowpool,
            tc.tile_pool(name="selpool", bufs=2) as selpool,
            tc.tile_pool(name="outpool", bufs=3) as outpool,
            tc.tile_pool(name="ps_small", bufs=2, space="PSUM") as ps_small_pool,
            tc.tile_pool(name="ps_out", bufs=2, space="PSUM") as ps_out_pool,
        ):
            ident = constp.tile([L, L], f32, tag="ident")
            nc.sync.dma_start(ident[:], ident_d[:, :])
            ones1 = constp.tile([1, L], f32, tag="ones1")
            nc.sync.dma_start(ones1[:], ones_d[:, :])
            e127 = constp.tile([1, L], f32, tag="e127")
            nc.sync.dma_start(e127[:], e127_d[:, :])

            prev = None
            for k in range(nb):
                r0 = k * L
                tin = inpool.tile([L, F], f32, tag="tin")
                nc.sync.dma_start(tin[:], inp[r0 : r0 + L, :])

                # y_t of this block as a [1, L] row (PE transpose), then
                # broadcast across partitions via ones ⊗ y_row (rank-1 matmul)
                ps = ps_small_pool.tile([L, 3 * L], f32, tag="ps_small")
                yrow_p = ps[0:1, 0:L]
                ybc_p = ps[:, L : 2 * L]
                sel_p = ps[:, 2 * L : 3 * L]
                nc.tensor.transpose(yrow_p, tin[:, 1:2], ident[:])
                yrow = rowpool.tile([1, L], f32, tag="yrow")
                nc.scalar.copy(yrow[:], yrow_p)
                nc.tensor.matmul(ybc_p, ones1[:], yrow[:], start=True, stop=True)

                # mt[s, t] = M[t, s] = prod_{r=s+1..t} y_r (0 for t < s)
                mt = mpool.tile([L, L], f32, tag="mt")
                nc.vector.tensor_tensor_scan(
                    mt[:],
                    ybc_p,
                    ident[:],
                    0.0,
                    mybir.AluOpType.mult,
                    mybir.AluOpType.add,
                )

                # u[s, :] = x_s * z_s
                u = upool.tile([L, U], f32, tag="u")
                nc.vector.tensor_scalar_mul(u[:], tin[:, 2:F], tin[:, 0:1])

                po = ps_out_pool.tile([L, U], f32, tag="po")
                if k > 0:
                    # p_t = prod_{r=block_start..t} y_r = y_0 * mt[0, t]
                    prow = rowpool.tile([1, L], f32, tag="prow")
                    nc.vector.tensor_scalar_mul(prow[:], mt[0:1, :], yrow[0:1, 0:1])
                    # sel[s, t] = I[s==127] * p_t
                    nc.tensor.matmul(sel_p, e127[:], prow[:], start=True, stop=True)
                    sel = selpool.tile([L, L], f32, tag="sel")
                    nc.vector.tensor_copy(sel[:], sel_p)
                for j in (0, 512):
                    nc.tensor.matmul(
                        po[:, j : j + 512],
                        mt[:],
                        u[:, j : j + 512],
                        start=True,
                        stop=(k == 0),
                    )
                if k > 0:
                    # po[t, :] += p_t * prev[127, :]
                    for j in (0, 512):
                        nc.tensor.matmul(
                            po[:, j : j + 512],
                            sel[:],
                            prev[:, j : j + 512],
                            start=False,
                            stop=True,
                        )
                ot = outpool.tile([L, U], f32, tag="ot")
                nc.scalar.copy(ot[:], po[:])
                nc.sync.dma_start(out[r0 : r0 + L, :], ot[:])
                prev = ot
    nc.finalize()
    return nc


_NC = None


def _get_nc() -> bass.Bass:
    global _NC
    if _NC is None:
        _NC = build_nc()
    return _NC


def kernel(**inputs: np.ndarray) -> np.ndarray:
    x = np.ascontiguousarray(inputs["inputs"], dtype=np.float32)
    assert x.shape == (B, T, F), x.shape
    nc = _get_nc()
    in_maps = [{"inp": x[c]} for c in range(B)]
    res = run_bass_kernel_spmd(nc, in_maps, core_ids=list(range(B)))
    return np.stack([res.results[c]["out"] for c in range(B)], axis=0)


# revision 8
# speedup vs baseline: 1.0945x; 1.0945x over previous
"""Trainium2 Bass kernel for CLSProcess: diagonal linear recurrence
state_t = y_t * state_{t-1} + x_t * z_t over [B=8, T=4096, units=1024].

Sharding: batch across the 8 cores (one batch element per core); the
recurrence is handled per-core with a chunked scan:
  - time is cut into 32 blocks of L=128 steps (partition dim = time)
  - per block, the decay matrix M[t,s] = prod_{r=s+1..t} y_r (0 for s>t)
    is built EXACTLY (no log/exp) with one DVE tensor_tensor_scan over the
    identity matrix: state_s(t) = y_t*state + I[s==t]  =>  out[s,t] = M[t,s],
    which is already the lhsT layout the PE matmul wants
  - block output = M @ (x*z)  (PE matmul, fp32)  +  carry term
  - carry term: engines can only address partition bases {0,32,64,96}, so
    instead of extracting row 127 of the previous block we build
    sel[s,t] = I[s==127] * p_t  (rank-1 PE matmul e127 ⊗ p_row, where
    p_t = prod_{r=block_start..t} y_r = y_0 * M[t,0]) and accumulate
    sel^T @ prev_out into the same PSUM, which equals p_t * prev_state.
"""

import numpy as np

import concourse.bacc as bacc
import concourse.bass as bass
import concourse.mybir as mybir
import concourse.tile as tile
from concourse.bass_utils import run_bass_kernel_spmd

B = 8
T = 4096
F = 1026
U = 1024
L = 128
f32 = mybir.dt.float32
f32r = mybir.dt.float32r


def build_nc(t_total: int = T) -> bass.Bass:
    nb = t_total // L
    nc = bacc.Bacc()
    inp = nc.dram_tensor("inp", [t_total, F], f32, kind="ExternalInput")
    out = nc.dram_tensor("out", [t_total, U], f32, kind="ExternalOutput")
    ident_d = nc.inline_tensor(np.eye(L, dtype=np.float32), name="ident")
    ones_d = nc.inline_tensor(np.ones((1, L), dtype=np.float32), name="ones1")
    e127_np = np.zeros((1, L), dtype=np.float32)
    e127_np[0, L - 1] = 1.0
    e127_d = nc.inline_tensor(e127_np, name="e127")

    with tile.TileContext(nc) as tc:
        with (
            tc.tile_pool(name="const", bufs=1) as constp,
            tc.tile_pool(name="inpool", bufs=4) as inpool,
            tc.tile_pool(name="upool", bufs=2) as upool,
            tc.tile_pool(name="mpool", bufs=2) as mpool,
            tc.tile_pool(name="rowpool", bufs=2) as rowpool,
            tc.tile_pool(name="selpool", bufs=2) as selpool,
            tc.tile_pool(name="outpool", bufs=3) as outpool,
            tc.tile_pool(name="ps_small", bufs=2, space="PSUM") as ps_small_pool,
            tc.tile_pool(name="ps_out", bufs=2, space="PSUM") as ps_out_pool,
        ):
            ident = constp.tile([L, L], f32, tag="ident")
            nc.sync.dma_start(ident[:], ident_d[:, :])
            ones1 = constp.tile([1, L], f32, tag="ones1")
            nc.sync.dma_start(ones1[:], ones_d[:, :])
            e127 = constp.tile([1, L], f32, tag="e127")
            nc.sync.dma_start(e127[:], e127_d[:, :])

            prev = None
            for k in range(nb):
                r0 = k * L
                tin = inpool.tile([L, F], f32, tag="tin")
                nc.sync.dma_start(tin[:], inp[r0 : r0 + L, :])

                # y_t of this block as a [1, L] row (PE transpose), then
                # broadcast across partitions via ones ⊗ y_row (rank-1 matmul)
                ps = ps_small_pool.tile([L, 3 * L], f32, tag="ps_small")
                yrow_p = ps[0:1, 0:L]
                ybc_p = ps[:, L : 2 * L]
                sel_p = ps[:, 2 * L : 3 * L]
                nc.tensor.transpose(yrow_p, tin[:, 1:2], ident[:])
                yrow = rowpool.tile([1, L], f32, tag="yrow")
                nc.scalar.copy(yrow[:], yrow_p)
                nc.tensor.matmul(ybc_p, ones1[:], yrow[:], start=True, stop=True)

                # mt[s, t] = M[t, s] = prod_{r=s+1..t} y_r (0 for t < s)
                mt = mpool.tile([L, L], f32r, tag="mt")
                nc.vector.tensor_tensor_scan(
                    mt[:],
                    ybc_p,
                    ident[:],
                    0.0,
                    mybir.AluOpType.mult,
                    mybir.AluOpType.add,
                )

                # u[s, :] = x_s * z_s
                u = upool.tile([L, U], f32r, tag="u")
                nc.vector.tensor_scalar_mul(u[:], tin[:, 2:F], tin[:, 0:1])

                po = ps_out_pool.tile([L, U], f32, tag="po")
                if k > 0:
                    # p_t = prod_{r=block_start..t} y_r = y_0 * mt[0, t]
                    prow = rowpool.tile([1, L], f32, tag="prow")
                    nc.vector.tensor_scalar_mul(prow[:], mt[0:1, :], yrow[0:1, 0:1])
                    # sel[s, t] = I[s==127] * p_t
                    nc.tensor.matmul(sel_p, e127[:], prow[:], start=True, stop=True)
                    sel = selpool.tile([L, L], f32r, tag="sel")
                    nc.vector.tensor_copy(sel[:], sel_p)
                # float32r = single-pass (reduced-precision) fp32 on the PE —
                # halves matmul cost vs the LOW_HIGH double-pass fp32 mode
                for j in (0, 512):
                    nc.tensor.matmul(
                        po[:, j : j + 512],
                        mt[:],
                        u[:, j : j + 512],
                        start=True,
                        stop=(k == 0),
                    )
                if k > 0:
                    # po[t, :] += p_t * prev[127, :]
                    for j in (0, 512):
                        nc.tensor.matmul(
                            po[:, j : j + 512],
                            sel[:],
                            prev[:, j : j + 512],
                            start=False,
                            stop=True,
                        )
                ot = outpool.tile([L, U], f32r, tag="ot")
                nc.scalar.copy(ot[:], po[:])
                nc.sync.dma_start(out[r0 : r0 + L, :], ot[:].bitcast(f32))
                prev = ot
    nc.finalize()
    return nc


_NC = None


def _get_nc() -> bass.Bass:
    global _NC
    if _NC is None:
        _NC = build_nc()
    return _NC


def kernel(**inputs: np.ndarray) -> np.ndarray:
    x = np.ascontiguousarray(inputs["inputs"], dtype=np.float32)
    assert x.shape == (B, T, F), x.shape
    nc = _get_nc()
    in_maps = [{"inp": x[c]} for c in range(B)]
    res = run_bass_kernel_spmd(nc, in_maps, core_ids=list(range(B)))
    return np.stack([res.results[c]["out"] for c in range(B)], axis=0)


# revision 9
# speedup vs baseline: 1.3088x; 1.1958x over previous
"""Trainium2 Bass kernel for CLSProcess: diagonal linear recurrence
state_t = y_t * state_{t-1} + x_t * z_t over [B=8, T=4096, units=1024].

Sharding: batch across the 8 cores (one batch element per core); the
recurrence is handled per-core with a chunked scan:
  - time is cut into 32 blocks of L=128 steps (partition dim = time)
  - per block, the decay matrix M[t,s] = prod_{r=s+1..t} y_r (0 for s>t)
    is built EXACTLY (no log/exp) with one DVE tensor_tensor_scan over the
    identity matrix: state_s(t) = y_t*state + I[s==t]  =>  out[s,t] = M[t,s],
    which is already the lhsT layout the PE matmul wants
  - block output = M @ (x*z)  (PE matmul, float32r single-pass)  + carry term
  - carry term: engines can only address partition bases {0,32,64,96}, so
    instead of extracting row 127 of the previous block we build
    sel[s,t] = I[s==127] * p_t  (p_t = prod_{r=block_start..t} y_r
    = y_0 * M[t,0], broadcast via GPSIMD + mask on DVE) and accumulate
    sel^T @ prev_out into the same PSUM, which equals p_t * prev_state.
  - the y-row broadcast for the scan runs on the (otherwise idle) GPSIMD.
"""

import numpy as np

import concourse.bacc as bacc
import concourse.bass as bass
import concourse.mybir as mybir
import concourse.tile as tile
from concourse.bass_utils import run_bass_kernel_spmd

B = 8
T = 4096
F = 1026
U = 1024
L = 128
f32 = mybir.dt.float32
f32r = mybir.dt.float32r


def build_nc(t_total: int = T) -> bass.Bass:
    nb = t_total // L
    nc = bacc.Bacc()
    inp = nc.dram_tensor("inp", [t_total, F], f32, kind="ExternalInput")
    out = nc.dram_tensor("out", [t_total, U], f32, kind="ExternalOutput")
    ident_d = nc.inline_tensor(np.eye(L, dtype=np.float32), name="ident")
    e127c_np = np.zeros((L, 1), dtype=np.float32)
    e127c_np[L - 1, 0] = 1.0
    e127c_d = nc.inline_tensor(e127c_np, name="e127c")

    with tile.TileContext(nc) as tc:
        with (
            tc.tile_pool(name="const", bufs=1) as constp,
            tc.tile_pool(name="inpool", bufs=4) as inpool,
            tc.tile_pool(name="upool", bufs=2) as upool,
            tc.tile_pool(name="mpool", bufs=2) as mpool,
            tc.tile_pool(name="rowpool", bufs=2) as rowpool,
            tc.tile_pool(name="bcpool", bufs=2) as bcpool,
            tc.tile_pool(name="selpool", bufs=2) as selpool,
            tc.tile_pool(name="outpool", bufs=3) as outpool,
            tc.tile_pool(name="ps_small", bufs=2, space="PSUM") as ps_small_pool,
            tc.tile_pool(name="ps_out", bufs=2, space="PSUM") as ps_out_pool,
        ):
            ident = constp.tile([L, L], f32, tag="ident")
            nc.sync.dma_start(ident[:], ident_d[:, :])
            e127c = constp.tile([L, 1], f32, tag="e127c")
            nc.sync.dma_start(e127c[:], e127c_d[:, :])

            prev = None
            for k in range(nb):
                r0 = k * L
                tin = inpool.tile([L, F], f32, tag="tin")
                nc.sync.dma_start(tin[:], inp[r0 : r0 + L, :])

                # y_t of this block as a [1, L] row (PE transpose), then
                # broadcast across partitions on GPSIMD
                ps = ps_small_pool.tile([1, L], f32, tag="ps_small")
                nc.tensor.transpose(ps[0:1, :], tin[:, 1:2], ident[:])
                yrow = rowpool.tile([1, L], f32, tag="yrow")
                nc.scalar.copy(yrow[:], ps[0:1, :])
                ybc = bcpool.tile([L, L], f32, tag="ybc")
                nc.gpsimd.partition_broadcast(ybc[:], yrow[0:1, :])

                # mt[s, t] = M[t, s] = prod_{r=s+1..t} y_r (0 for t < s)
                mt = mpool.tile([L, L], f32r, tag="mt")
                nc.vector.tensor_tensor_scan(
                    mt[:],
                    ybc[:],
                    ident[:],
                    0.0,
                    mybir.AluOpType.mult,
                    mybir.AluOpType.add,
                )

                # u[s, :] = x_s * z_s
                u = upool.tile([L, U], f32r, tag="u")
                nc.vector.tensor_scalar_mul(u[:], tin[:, 2:F], tin[:, 0:1])

                po = ps_out_pool.tile([L, U], f32, tag="po")
                if k > 0:
                    # p_t = prod_{r=block_start..t} y_r = y_0 * mt[0, t]
                    prow = rowpool.tile([1, L], f32, tag="prow")
                    nc.vector.tensor_scalar_mul(prow[:], mt[0:1, :], yrow[0:1, 0:1])
                    pbc = bcpool.tile([L, L], f32, tag="pbc")
                    nc.gpsimd.partition_broadcast(pbc[:], prow[0:1, :])
                    # sel[s, t] = I[s==127] * p_t
                    sel = selpool.tile([L, L], f32r, tag="sel")
                    nc.vector.tensor_scalar_mul(sel[:], pbc[:], e127c[:])
                # float32r = single-pass (reduced-precision) fp32 on the PE
                for j in (0, 512):
                    nc.tensor.matmul(
                        po[:, j : j + 512],
                        mt[:],
                        u[:, j : j + 512],
                        start=True,
                        stop=(k == 0),
                    )
                if k > 0:
                    # po[t, :] += p_t * prev[127, :]
                    for j in (0, 512):
                        nc.tensor.matmul(
                            po[:, j : j + 512],
                            sel[:],
                            prev[:, j : j + 512],
                            start=False,
                            stop=True,
                        )
                ot = outpool.tile([L, U], f32r, tag="ot")
                nc.scalar.copy(ot[:], po[:])
                nc.sync.dma_start(out[r0 : r0 + L, :], ot[:].bitcast(f32))
                prev = ot
    nc.finalize()
    return nc


_NC = None


def _get_nc() -> bass.Bass:
    global _NC
    if _NC is None:
        _NC = build_nc()
    return _NC


def kernel(**inputs: np.ndarray) -> np.ndarray:
    x = np.ascontiguousarray(inputs["inputs"], dtype=np.float32)
    assert x.shape == (B, T, F), x.shape
    nc = _get_nc()
    in_maps = [{"inp": x[c]} for c in range(B)]
    res = run_bass_kernel_spmd(nc, in_maps, core_ids=list(range(B)))
    return np.stack([res.results[c]["out"] for c in range(B)], axis=0)
